# revision 12
# baseline (speedup 1.0000x reference)
"""Combined CE + Dice loss on 8 Trainium2 NeuronCores (Bass/Tile).

Strategy (data-parallel over batch, 2 images per core):
  - Host: shard batch, repack inputs class-major [C, NPIX] contiguous,
    targets as float (values 0..8 exact), per-class counts via bincount.
  - Device (per core), tiles of [C*BPT rows, F cols] where row=(c, blk):
      ACT : E = exp(X)
      PE  : S[blk, f] = sum_c E[(c,blk), f]           (block-selector matmul)
      DVE : R = 1/S
      DMA : broadcast R and T across the 9 class rows
      DVE : P = E * Rb           (+ per-row sums -> sum_probs partials)
      DVE : Dm = (Tb==c) * P     (+ per-row sums -> intersection partials)
      PE  : G[blk, f] = sum_c Dm                       (= prob at target)
      ACT : ln(G) with accum     (-> CE partials)
  - Host: combine partials -> CE mean, dice terms -> scalar loss.
"""

import os
import sys
import numpy as np

for _p in ("/opt/trn_rl_repo",):
    if _p not in sys.path and os.path.isdir(_p):
        sys.path.insert(0, _p)

os.environ.setdefault("NEURON_RT_RESET_CORES", "1")

import concourse.bass as bass
import concourse.bacc as bacc
import concourse.tile as tile
from concourse import mybir
from concourse.bass_utils import run_bass_kernel_spmd

# ---------------- problem constants ----------------
B, C, H, W = 16, 9, 512, 512
HW = H * W                      # 262144 pixels per image
NCORES = 8
B_LOC = B // NCORES             # 2 images per core
NPIX = B_LOC * HW               # 524288 pixels per core

CE_WEIGHT = 0.7
DICE_WEIGHT = 0.3
EPS = 1e-5

# ---------------- tiling constants -----------------
F = 2048                        # pixels per block (free dim)
NBLK = NPIX // F                # 512 blocks per core
BPT = 14                        # blocks per full tile (9*14=126 partitions)
NFULL = NBLK // BPT             # 36 full tiles
REM = NBLK - NFULL * BPT        # 8 blocks in the tail tile
TILES_PER_GROUP = 9             # full tiles per packed group (9*14=126 rows)
NGRP_FULL = NFULL // TILES_PER_GROUP  # 4
NT = NFULL + (1 if REM else 0)  # accumulator columns (37)
NGRP = NGRP_FULL + (1 if REM else 0)  # 5

F32 = mybir.dt.float32
XDT = mybir.dt.bfloat16         # dtype of x / E / P / Dm on device
TDT = mybir.dt.bfloat16         # dtype of broadcast targets + cvec

_NP_OF = {mybir.dt.float32: np.float32, mybir.dt.bfloat16: np.float32}


def _np_dt(dt):
    import ml_dtypes
    if dt == mybir.dt.float32:
        return np.float32
    if dt == mybir.dt.bfloat16:
        return ml_dtypes.bfloat16
    raise ValueError(dt)


# ---------------- host-side constants ----------------
def _make_consts():
    # bselbig[:, j, :]: maps tile j of a 9-tile group into rows 14j..14j+13
    bselbig = np.zeros((C * BPT, TILES_PER_GROUP, C * BPT), dtype=np.float32)
    for j in range(TILES_PER_GROUP):
        for c in range(C):
            for b in range(BPT):
                bselbig[c * BPT + b, j, j * BPT + b] = 1.0
    bselbig = bselbig.reshape(C * BPT, TILES_PER_GROUP * C * BPT)
    cvec14 = np.repeat(np.arange(C, dtype=np.float32), BPT)[:, None]
    if REM:
        bsel_s = np.zeros((C * REM, REM), dtype=np.float32)
        for c in range(C):
            for b in range(REM):
                bsel_s[c * REM + b, b] = 1.0
        cvec_s = np.repeat(np.arange(C, dtype=np.float32), REM)[:, None]
    else:
        bsel_s = np.zeros((1, 1), np.float32)
        cvec_s = np.zeros((1, 1), np.float32)
    return bselbig, cvec14, bsel_s, cvec_s


# ---------------- device program ----------------
def build_program():
    nc = bacc.Bacc()

    x = nc.declare_dram_parameter("x", [C, NPIX], XDT, isOutput=False).ap()
    t = nc.declare_dram_parameter("t", [NPIX], TDT, isOutput=False).ap()
    bselbig_d = nc.declare_dram_parameter("bselbig", [C * BPT, TILES_PER_GROUP * C * BPT], XDT, isOutput=False).ap()
    cvec14_d = nc.declare_dram_parameter("cvec14", [C * BPT, 1], TDT, isOutput=False).ap()
    bsel_s_d = nc.declare_dram_parameter("bsel_s", [max(C * REM, 1), max(REM, 1)], XDT, isOutput=False).ap()
    cvec_s_d = nc.declare_dram_parameter("cvec_s", [max(C * REM, 1), 1], TDT, isOutput=False).ap()

    aacc_d = nc.declare_dram_parameter("aacc", [C * BPT, NT], F32, isOutput=True).ap()
    dacc_d = nc.declare_dram_parameter("dacc", [C * BPT, NT], F32, isOutput=True).ap()
    ceacc_d = nc.declare_dram_parameter("ceacc", [C * BPT, NGRP], F32, isOutput=True).ap()

    # groups: (list of global tile ids, blocks-per-tile, bsel handle-id)
    groups = []
    for g in range(NGRP_FULL):
        groups.append((list(range(g * TILES_PER_GROUP, (g + 1) * TILES_PER_GROUP)), BPT))
    if REM:
        groups.append(([NFULL], REM))

    from contextlib import ExitStack

    with tile.TileContext(nc) as tc, ExitStack() as ctx:
        consts = ctx.enter_context(tc.tile_pool(name="consts", bufs=1))
        xp = ctx.enter_context(tc.tile_pool(name="xp", bufs=3))
        ep = ctx.enter_context(tc.tile_pool(name="ep", bufs=TILES_PER_GROUP + 2))
        tbp = ctx.enter_context(tc.tile_pool(name="tbp", bufs=3))
        rbp = ctx.enter_context(tc.tile_pool(name="rbp", bufs=3))
        pp = ctx.enter_context(tc.tile_pool(name="pp", bufs=3))
        dmp = ctx.enter_context(tc.tile_pool(name="dmp", bufs=3))
        rp = ctx.enter_context(tc.tile_pool(name="rp", bufs=2))
        lnp = ctx.enter_context(tc.tile_pool(name="lnp", bufs=2))
        sps = ctx.enter_context(tc.tile_pool(name="sps", bufs=1, space="PSUM"))
        gps = ctx.enter_context(tc.tile_pool(name="gps", bufs=1, space="PSUM"))

        if True:
            bbig = consts.tile([C * BPT, TILES_PER_GROUP * C * BPT], XDT)
            nc.gpsimd.dma_start(out=bbig, in_=bselbig_d)
            cv14 = consts.tile([C * BPT, 1], TDT)
            nc.gpsimd.dma_start(out=cv14, in_=cvec14_d)
            if REM:
                bs = consts.tile([C * REM, REM], XDT)
                nc.gpsimd.dma_start(out=bs, in_=bsel_s_d)
                cvs = consts.tile([C * REM, 1], TDT)
                nc.gpsimd.dma_start(out=cvs, in_=cvec_s_d)

            aacc = consts.tile([C * BPT, NT], F32)
            dacc = consts.tile([C * BPT, NT], F32)
            ceacc = consts.tile([C * BPT, NGRP], F32)
            nc.vector.memset(aacc, 0.0)
            nc.vector.memset(dacc, 0.0)
            nc.vector.memset(ceacc, 0.0)

            NCHUNK = F // 512

            for g, (tile_ids, bpt) in enumerate(groups):
                rows = C * bpt               # 126 or 72
                srows = len(tile_ids) * bpt  # 126 or 8
                cvec = cv14 if bpt == BPT else cvs

                spack = sps.tile([C * BPT, F], F32)

                # phase 1: load, exp, pack sumexp into PSUM
                ets = []
                for jj, tid in enumerate(tile_ids):
                    xsl = x[:, tid * BPT * F: tid * BPT * F + bpt * F]
                    xv = xsl.rearrange("c (b f) -> c b f", f=F)
                    xt = xp.tile([C * BPT, F], XDT)
                    nc.gpsimd.dma_start(out=xt[:rows], in_=xv)

                    et = ep.tile([C * BPT, F], XDT)
                    nc.scalar.activation(
                        out=et[:rows], in_=xt[:rows],
                        func=mybir.ActivationFunctionType.Exp,
                    )
                    ets.append(et)

                    nacc = len(tile_ids)
                    for k in range(NCHUNK):
                        cs = slice(k * 512, (k + 1) * 512)
                        if bpt == BPT:
                            nc.tensor.matmul(
                                out=spack[:C * BPT, cs],
                                lhsT=bbig[:, jj * C * BPT:(jj + 1) * C * BPT],
                                rhs=et[:rows, cs],
                                start=(jj == 0), stop=(jj == nacc - 1),
                            )
                        else:
                            nc.tensor.matmul(
                                out=spack[:REM, cs],
                                lhsT=bs[:rows, :REM],
                                rhs=et[:rows, cs],
                                start=True, stop=True,
                            )

                # R = 1/S for the whole packed group (bf16 out is plenty here)
                rpk = rp.tile([C * BPT, F], XDT)
                with nc.allow_low_precision(reason="R rounding averages out over 2k-px sums"):
                    nc.vector.reciprocal(out=rpk[:srows], in_=spack[:srows])

                gpack = gps.tile([C * BPT, F], F32)

                # phase 2: broadcast, normalize, mask, reduce
                for jj, tid in enumerate(tile_ids):
                    # broadcast targets across the 9 class rows (from HBM)
                    tsl = t[tid * BPT * F: tid * BPT * F + bpt * F]
                    tv = tsl.rearrange("(b f) -> b f", f=F)
                    tbc = bass.AP(tensor=tv.tensor, offset=tv.offset,
                                  ap=[[0, C]] + list(tv.ap))
                    tbt = tbp.tile([C * BPT, F], TDT)
                    nc.scalar.dma_start(out=tbt[:rows], in_=tbc)

                    # broadcast R rows for this tile across class rows (SBUF->SBUF)
                    rsl = rpk[jj * bpt:(jj + 1) * bpt, :]
                    rbt = rbp.tile([C * BPT, F], XDT)
                    for c in range(C):
                        nc.gpsimd.dma_start(
                            out=rbt[c * bpt:(c + 1) * bpt, :], in_=rsl)

                    # P = E * Rb ; accum -> sum_probs partials
                    pt = pp.tile([C * BPT, F], XDT)
                    nc.vector.scalar_tensor_tensor(
                        out=pt[:rows], in0=ets[jj][:rows], scalar=0.0,
                        in1=rbt[:rows],
                        op0=mybir.AluOpType.bypass, op1=mybir.AluOpType.mult,
                        accum_out=aacc[:rows, tid:tid + 1],
                    )

                    # Dm = (Tb == c) * P ; accum -> intersection partials
                    dmt = dmp.tile([C * BPT, F], XDT)
                    nc.vector.scalar_tensor_tensor(
                        out=dmt[:rows], in0=tbt[:rows], scalar=cvec[:rows],
                        in1=pt[:rows],
                        op0=mybir.AluOpType.is_equal, op1=mybir.AluOpType.mult,
                        accum_out=dacc[:rows, tid:tid + 1],
                    )

                    # G = sum_c Dm  (prob at target), packed like S
                    for k in range(NCHUNK):
                        cs = slice(k * 512, (k + 1) * 512)
                        if bpt == BPT:
                            nc.tensor.matmul(
                                out=gpack[:C * BPT, cs],
                                lhsT=bbig[:, jj * C * BPT:(jj + 1) * C * BPT],
                                rhs=dmt[:rows, cs],
                                start=(jj == 0), stop=(jj == len(tile_ids) - 1),
                            )
                        else:
                            nc.tensor.matmul(
                                out=gpack[:REM, cs],
                                lhsT=bs[:rows, :REM],
                                rhs=dmt[:rows, cs],
                                start=True, stop=True,
                            )

                # CE partials: sum of ln(G) over the group
                lnt = lnp.tile([C * BPT, F], F32)
                nc.scalar.activation(
                    out=lnt[:srows], in_=gpack[:srows],
                    func=mybir.ActivationFunctionType.Ln,
                    accum_out=ceacc[:srows, g:g + 1],
                )

            nc.gpsimd.dma_start(out=aacc_d, in_=aacc)
            nc.gpsimd.dma_start(out=dacc_d, in_=dacc)
            nc.gpsimd.dma_start(out=ceacc_d, in_=ceacc)

    if not nc.is_finalized():
        nc.finalize()
    return nc


_NC_CACHE = None


def _get_nc():
    global _NC_CACHE
    if _NC_CACHE is None:
        _NC_CACHE = build_program()
    return _NC_CACHE


# ---------------- host side ----------------
def _prep_in_maps(inputs, targets):
    x = np.asarray(inputs, dtype=np.float32).reshape(B, C, HW)
    t = np.asarray(targets).reshape(B, HW)
    bselbig, cvec14, bsel_s, cvec_s = _make_consts()
    xdt = _np_dt(XDT)
    tdt = _np_dt(TDT)
    in_maps = []
    for core in range(NCORES):
        xs = x[core * B_LOC:(core + 1) * B_LOC]          # [B_LOC, C, HW]
        xs_cm = np.ascontiguousarray(xs.transpose(1, 0, 2)).reshape(C, NPIX)
        ts = t[core * B_LOC:(core + 1) * B_LOC].reshape(NPIX)
        in_maps.append({
            "x": xs_cm.astype(xdt, copy=False),
            "t": ts.astype(tdt),
            "bselbig": bselbig.astype(xdt),
            "cvec14": cvec14.astype(tdt),
            "bsel_s": bsel_s.astype(xdt),
            "cvec_s": cvec_s.astype(tdt),
        })
    return in_maps


def _combine(results, targets):
    """Map per-core per-(row, tile) partials to per-(image, class) sums."""
    t = np.asarray(targets).reshape(B, HW)

    A = np.zeros((B, C), dtype=np.float64)   # sum of probs
    D = np.zeros((B, C), dtype=np.float64)   # intersection
    ce_sum = 0.0

    blk_per_img = HW // F                    # blocks per image

    for core in range(NCORES):
        aacc = np.asarray(results[core]["aacc"], dtype=np.float64)
        dacc = np.asarray(results[core]["dacc"], dtype=np.float64)
        ceacc = np.asarray(results[core]["ceacc"], dtype=np.float64)

        # full tiles
        for tid in range(NFULL):
            for p in range(C * BPT):
                c, b = divmod(p, BPT)
                blk = tid * BPT + b
                img = core * B_LOC + blk // blk_per_img
                A[img, c] += aacc[p, tid]
                D[img, c] += dacc[p, tid]
        # tail tile
        if REM:
            for p in range(C * REM):
                c, b = divmod(p, REM)
                blk = NFULL * BPT + b
                img = core * B_LOC + blk // blk_per_img
                A[img, c] += aacc[p, NFULL]
                D[img, c] += dacc[p, NFULL]

        # CE: group g covers rows 0..srows-1
        for g in range(NGRP_FULL):
            ce_sum += ceacc[:C * BPT, g].sum()
        if REM:
            ce_sum += ceacc[:REM, NGRP_FULL].sum()

    # one-hot counts, exact on host
    Bcnt = np.zeros((B, C), dtype=np.float64)
    for img in range(B):
        Bcnt[img] = np.bincount(t[img].astype(np.int64), minlength=C)[:C]

    ce_loss = -ce_sum / (B * HW)

    card = A + Bcnt
    dice = np.where(card > 0, 2.0 * D / (card + EPS), 1.0)
    dice_loss = 1.0 - dice.mean()

    return np.float32(CE_WEIGHT * ce_loss + DICE_WEIGHT * dice_loss)


def _run_hw(in_maps, trace=False):
    nc = _get_nc()
    res = run_bass_kernel_spmd(nc, in_maps, list(range(NCORES)), trace=trace)
    return res


def _run_sim(in_maps):
    from concourse import bass_interp
    nc = _get_nc()
    results = []
    for core in range(NCORES):
        sim = bass_interp.CoreSim(nc)
        for k, v in in_maps[core].items():
            sim.tensor(k)[:] = v
        sim.simulate()
        results.append({k: np.array(sim.tensor(k))
                        for k in ("aacc", "dacc", "ceacc")})
    return results


def kernel(inputs, targets):
    in_maps = _prep_in_maps(inputs, targets)
    if os.environ.get("CEDICE_SIM"):
        results = _run_sim(in_maps)
    else:
        results = _run_hw(in_maps).results
    return _combine(results, targets)


# revision 14
# speedup vs baseline: 1.0475x; 1.0475x over previous
"""Combined CE + Dice loss on 8 Trainium2 NeuronCores (Bass/Tile).

Strategy (data-parallel over batch, 2 images per core):
  - Host: shard batch, repack inputs class-major [C, NPIX] contiguous,
    targets as float (values 0..8 exact), per-class counts via bincount.
  - Device (per core), tiles of [C*BPT rows, F cols] where row=(c, blk):
      ACT : E = exp(X)
      PE  : S[blk, f] = sum_c E[(c,blk), f]           (block-selector matmul)
      DVE : R = 1/S
      DMA : broadcast R and T across the 9 class rows
      DVE : P = E * Rb           (+ per-row sums -> sum_probs partials)
      DVE : Dm = (Tb==c) * P     (+ per-row sums -> intersection partials)
      PE  : G[blk, f] = sum_c Dm                       (= prob at target)
      ACT : ln(G) with accum     (-> CE partials)
  - Host: combine partials -> CE mean, dice terms -> scalar loss.
"""

import os
import sys
import numpy as np

for _p in ("/opt/trn_rl_repo",):
    if _p not in sys.path and os.path.isdir(_p):
        sys.path.insert(0, _p)

os.environ.setdefault("NEURON_RT_RESET_CORES", "1")

import concourse.bass as bass
import concourse.bacc as bacc
import concourse.tile as tile
from concourse import mybir
from concourse.bass_utils import run_bass_kernel_spmd

# ---------------- problem constants ----------------
B, C, H, W = 16, 9, 512, 512
HW = H * W                      # 262144 pixels per image
NCORES = 8
B_LOC = B // NCORES             # 2 images per core
NPIX = B_LOC * HW               # 524288 pixels per core

CE_WEIGHT = 0.7
DICE_WEIGHT = 0.3
EPS = 1e-5

# ---------------- tiling constants -----------------
F = 2048                        # pixels per block (free dim)
NBLK = NPIX // F                # 512 blocks per core
BPT = 14                        # blocks per full tile (9*14=126 partitions)
NFULL = NBLK // BPT             # 36 full tiles
REM = NBLK - NFULL * BPT        # 8 blocks in the tail tile
TILES_PER_GROUP = 9             # full tiles per packed group (9*14=126 rows)
NGRP_FULL = NFULL // TILES_PER_GROUP  # 4
NT = NFULL + (1 if REM else 0)  # accumulator columns (37)
NGRP = NGRP_FULL + (1 if REM else 0)  # 5

F32 = mybir.dt.float32
XDT = mybir.dt.bfloat16         # dtype of x / E / P / Dm on device
TDT = mybir.dt.uint8            # dtype of broadcast targets + cvec

_NP_OF = {mybir.dt.float32: np.float32, mybir.dt.bfloat16: np.float32}


def _np_dt(dt):
    import ml_dtypes
    if dt == mybir.dt.float32:
        return np.float32
    if dt == mybir.dt.bfloat16:
        return ml_dtypes.bfloat16
    if dt == mybir.dt.uint8:
        return np.uint8
    raise ValueError(dt)


# ---------------- host-side constants ----------------
def _make_consts():
    # bselbig[:, j, :]: maps tile j of a 9-tile group into rows 14j..14j+13
    bselbig = np.zeros((C * BPT, TILES_PER_GROUP, C * BPT), dtype=np.float32)
    for j in range(TILES_PER_GROUP):
        for c in range(C):
            for b in range(BPT):
                bselbig[c * BPT + b, j, j * BPT + b] = 1.0
    bselbig = bselbig.reshape(C * BPT, TILES_PER_GROUP * C * BPT)
    cvec14 = np.repeat(np.arange(C, dtype=np.float32), BPT)[:, None]
    if REM:
        bsel_s = np.zeros((C * REM, REM), dtype=np.float32)
        for c in range(C):
            for b in range(REM):
                bsel_s[c * REM + b, b] = 1.0
        cvec_s = np.repeat(np.arange(C, dtype=np.float32), REM)[:, None]
    else:
        bsel_s = np.zeros((1, 1), np.float32)
        cvec_s = np.zeros((1, 1), np.float32)
    return bselbig, cvec14, bsel_s, cvec_s


# ---------------- device program ----------------
def build_program():
    nc = bacc.Bacc()

    x = nc.declare_dram_parameter("x", [C, NPIX], XDT, isOutput=False).ap()
    t = nc.declare_dram_parameter("t", [NPIX], TDT, isOutput=False).ap()
    bselbig_d = nc.declare_dram_parameter("bselbig", [C * BPT, TILES_PER_GROUP * C * BPT], XDT, isOutput=False).ap()
    cvec14_d = nc.declare_dram_parameter("cvec14", [C * BPT, 1], TDT, isOutput=False).ap()
    bsel_s_d = nc.declare_dram_parameter("bsel_s", [max(C * REM, 1), max(REM, 1)], XDT, isOutput=False).ap()
    cvec_s_d = nc.declare_dram_parameter("cvec_s", [max(C * REM, 1), 1], TDT, isOutput=False).ap()

    aacc_d = nc.declare_dram_parameter("aacc", [C * BPT, NT], F32, isOutput=True).ap()
    dacc_d = nc.declare_dram_parameter("dacc", [C * BPT, NT], F32, isOutput=True).ap()
    ceacc_d = nc.declare_dram_parameter("ceacc", [C * BPT, NGRP], F32, isOutput=True).ap()

    # groups: (list of global tile ids, blocks-per-tile, bsel handle-id)
    groups = []
    for g in range(NGRP_FULL):
        groups.append((list(range(g * TILES_PER_GROUP, (g + 1) * TILES_PER_GROUP)), BPT))
    if REM:
        groups.append(([NFULL], REM))

    from contextlib import ExitStack

    with tile.TileContext(nc) as tc, ExitStack() as ctx:
        consts = ctx.enter_context(tc.tile_pool(name="consts", bufs=1))
        xp = ctx.enter_context(tc.tile_pool(name="xp", bufs=3))
        ep = ctx.enter_context(tc.tile_pool(name="ep", bufs=TILES_PER_GROUP + 2))
        tbp = ctx.enter_context(tc.tile_pool(name="tbp", bufs=3))
        rbp = ctx.enter_context(tc.tile_pool(name="rbp", bufs=3))
        pp = ctx.enter_context(tc.tile_pool(name="pp", bufs=3))
        dmp = ctx.enter_context(tc.tile_pool(name="dmp", bufs=3))
        rp = ctx.enter_context(tc.tile_pool(name="rp", bufs=2))
        lnp = ctx.enter_context(tc.tile_pool(name="lnp", bufs=2))
        sps = ctx.enter_context(tc.tile_pool(name="sps", bufs=1, space="PSUM"))
        gps = ctx.enter_context(tc.tile_pool(name="gps", bufs=1, space="PSUM"))

        if True:
            bbig = consts.tile([C * BPT, TILES_PER_GROUP * C * BPT], XDT)
            nc.gpsimd.dma_start(out=bbig, in_=bselbig_d)
            cv14 = consts.tile([C * BPT, 1], TDT)
            nc.gpsimd.dma_start(out=cv14, in_=cvec14_d)
            if REM:
                bs = consts.tile([C * REM, REM], XDT)
                nc.gpsimd.dma_start(out=bs, in_=bsel_s_d)
                cvs = consts.tile([C * REM, 1], TDT)
                nc.gpsimd.dma_start(out=cvs, in_=cvec_s_d)

            aacc = consts.tile([C * BPT, NT], F32)
            dacc = consts.tile([C * BPT, NT], F32)
            ceacc = consts.tile([C * BPT, NGRP], F32)
            nc.vector.memset(aacc, 0.0)
            nc.vector.memset(dacc, 0.0)
            nc.vector.memset(ceacc, 0.0)

            NCHUNK = F // 512

            for g, (tile_ids, bpt) in enumerate(groups):
                rows = C * bpt               # 126 or 72
                srows = len(tile_ids) * bpt  # 126 or 8
                cvec = cv14 if bpt == BPT else cvs

                spack = sps.tile([C * BPT, F], F32)

                # phase 1: load, exp, pack sumexp into PSUM
                ets = []
                for jj, tid in enumerate(tile_ids):
                    xsl = x[:, tid * BPT * F: tid * BPT * F + bpt * F]
                    xv = xsl.rearrange("c (b f) -> c b f", f=F)
                    xt = xp.tile([C * BPT, F], XDT)
                    nc.gpsimd.dma_start(out=xt[:rows], in_=xv)

                    et = ep.tile([C * BPT, F], XDT)
                    nc.scalar.activation(
                        out=et[:rows], in_=xt[:rows],
                        func=mybir.ActivationFunctionType.Exp,
                    )
                    ets.append(et)

                    nacc = len(tile_ids)
                    for k in range(NCHUNK):
                        cs = slice(k * 512, (k + 1) * 512)
                        if bpt == BPT:
                            nc.tensor.matmul(
                                out=spack[:C * BPT, cs],
                                lhsT=bbig[:, jj * C * BPT:(jj + 1) * C * BPT],
                                rhs=et[:rows, cs],
                                start=(jj == 0), stop=(jj == nacc - 1),
                            )
                        else:
                            nc.tensor.matmul(
                                out=spack[:REM, cs],
                                lhsT=bs[:rows, :REM],
                                rhs=et[:rows, cs],
                                start=True, stop=True,
                            )

                # R = 1/S for the whole packed group (bf16 out is plenty here)
                rpk = rp.tile([C * BPT, F], XDT)
                with nc.allow_low_precision(reason="R rounding averages out over 2k-px sums"):
                    nc.vector.reciprocal(out=rpk[:srows], in_=spack[:srows])

                gpack = gps.tile([C * BPT, F], F32)

                # phase 2: broadcast, normalize, mask, reduce
                for jj, tid in enumerate(tile_ids):
                    # broadcast targets across the 9 class rows (from HBM)
                    tsl = t[tid * BPT * F: tid * BPT * F + bpt * F]
                    tv = tsl.rearrange("(b f) -> b f", f=F)
                    tbc = bass.AP(tensor=tv.tensor, offset=tv.offset,
                                  ap=[[0, C]] + list(tv.ap))
                    tbt = tbp.tile([C * BPT, F], TDT)
                    nc.scalar.dma_start(out=tbt[:rows], in_=tbc)

                    # broadcast R rows for this tile across class rows (SBUF->SBUF)
                    rsl = rpk[jj * bpt:(jj + 1) * bpt, :]
                    rbt = rbp.tile([C * BPT, F], XDT)
                    for c in range(C):
                        nc.gpsimd.dma_start(
                            out=rbt[c * bpt:(c + 1) * bpt, :], in_=rsl)

                    # P = E * Rb ; accum -> sum_probs partials
                    pt = pp.tile([C * BPT, F], XDT)
                    nc.vector.scalar_tensor_tensor(
                        out=pt[:rows], in0=ets[jj][:rows], scalar=0.0,
                        in1=rbt[:rows],
                        op0=mybir.AluOpType.bypass, op1=mybir.AluOpType.mult,
                        accum_out=aacc[:rows, tid:tid + 1],
                    )

                    # Dm = (Tb == c) * P ; accum -> intersection partials
                    dmt = dmp.tile([C * BPT, F], XDT)
                    nc.vector.scalar_tensor_tensor(
                        out=dmt[:rows], in0=tbt[:rows], scalar=cvec[:rows],
                        in1=pt[:rows],
                        op0=mybir.AluOpType.is_equal, op1=mybir.AluOpType.mult,
                        accum_out=dacc[:rows, tid:tid + 1],
                    )

                    # G = sum_c Dm  (prob at target), packed like S
                    for k in range(NCHUNK):
                        cs = slice(k * 512, (k + 1) * 512)
                        if bpt == BPT:
                            nc.tensor.matmul(
                                out=gpack[:C * BPT, cs],
                                lhsT=bbig[:, jj * C * BPT:(jj + 1) * C * BPT],
                                rhs=dmt[:rows, cs],
                                start=(jj == 0), stop=(jj == len(tile_ids) - 1),
                            )
                        else:
                            nc.tensor.matmul(
                                out=gpack[:REM, cs],
                                lhsT=bs[:rows, :REM],
                                rhs=dmt[:rows, cs],
                                start=True, stop=True,
                            )

                # CE partials: sum of ln(G) over the group
                lnt = lnp.tile([C * BPT, F], F32)
                nc.scalar.activation(
                    out=lnt[:srows], in_=gpack[:srows],
                    func=mybir.ActivationFunctionType.Ln,
                    accum_out=ceacc[:srows, g:g + 1],
                )

            nc.gpsimd.dma_start(out=aacc_d, in_=aacc)
            nc.gpsimd.dma_start(out=dacc_d, in_=dacc)
            nc.gpsimd.dma_start(out=ceacc_d, in_=ceacc)

    if not nc.is_finalized():
        nc.finalize()
    return nc


_NC_CACHE = None


def _get_nc():
    global _NC_CACHE
    if _NC_CACHE is None:
        _NC_CACHE = build_program()
    return _NC_CACHE


# ---------------- host side ----------------
def _prep_in_maps(inputs, targets):
    x = np.asarray(inputs, dtype=np.float32).reshape(B, C, HW)
    t = np.asarray(targets).reshape(B, HW)
    bselbig, cvec14, bsel_s, cvec_s = _make_consts()
    xdt = _np_dt(XDT)
    tdt = _np_dt(TDT)
    in_maps = []
    for core in range(NCORES):
        xs = x[core * B_LOC:(core + 1) * B_LOC]          # [B_LOC, C, HW]
        xs_cm = np.ascontiguousarray(xs.transpose(1, 0, 2)).reshape(C, NPIX)
        ts = t[core * B_LOC:(core + 1) * B_LOC].reshape(NPIX)
        in_maps.append({
            "x": xs_cm.astype(xdt, copy=False),
            "t": ts.astype(tdt),
            "bselbig": bselbig.astype(xdt),
            "cvec14": cvec14.astype(tdt),
            "bsel_s": bsel_s.astype(xdt),
            "cvec_s": cvec_s.astype(tdt),
        })
    return in_maps


def _combine(results, targets):
    """Map per-core per-(row, tile) partials to per-(image, class) sums."""
    t = np.asarray(targets).reshape(B, HW)

    A = np.zeros((B, C), dtype=np.float64)   # sum of probs
    D = np.zeros((B, C), dtype=np.float64)   # intersection
    ce_sum = 0.0

    blk_per_img = HW // F                    # blocks per image

    # row/tile -> (class, image-within-core) index maps, built once
    pf = np.arange(C * BPT)
    cf, bf = pf // BPT, pf % BPT             # full-tile row -> (c, b)
    tids = np.arange(NFULL)
    img_f = (tids[None, :] * BPT + bf[:, None]) // blk_per_img  # [rows, NFULL]
    if REM:
        ps = np.arange(C * REM)
        cs_, bs_ = ps // REM, ps % REM
        img_s = (NFULL * BPT + bs_) // blk_per_img

    for core in range(NCORES):
        aacc = np.asarray(results[core]["aacc"], dtype=np.float64)
        dacc = np.asarray(results[core]["dacc"], dtype=np.float64)
        ceacc = np.asarray(results[core]["ceacc"], dtype=np.float64)

        imgs = core * B_LOC + img_f          # [rows, NFULL]
        np.add.at(A, (imgs, np.broadcast_to(cf[:, None], imgs.shape)),
                  aacc[:C * BPT, :NFULL])
        np.add.at(D, (imgs, np.broadcast_to(cf[:, None], imgs.shape)),
                  dacc[:C * BPT, :NFULL])
        if REM:
            np.add.at(A, (core * B_LOC + img_s, cs_), aacc[:C * REM, NFULL])
            np.add.at(D, (core * B_LOC + img_s, cs_), dacc[:C * REM, NFULL])

        ce_sum += ceacc[:C * BPT, :NGRP_FULL].sum()
        if REM:
            ce_sum += ceacc[:REM, NGRP_FULL].sum()

    # one-hot counts, exact on host
    Bcnt = np.zeros((B, C), dtype=np.float64)
    for img in range(B):
        Bcnt[img] = np.bincount(t[img].astype(np.int64), minlength=C)[:C]

    ce_loss = -ce_sum / (B * HW)

    card = A + Bcnt
    dice = np.where(card > 0, 2.0 * D / (card + EPS), 1.0)
    dice_loss = 1.0 - dice.mean()

    return np.float32(CE_WEIGHT * ce_loss + DICE_WEIGHT * dice_loss)


def _run_hw(in_maps, trace=False):
    nc = _get_nc()
    res = run_bass_kernel_spmd(nc, in_maps, list(range(NCORES)), trace=trace)
    return res


def _run_sim(in_maps):
    from concourse import bass_interp
    nc = _get_nc()
    results = []
    for core in range(NCORES):
        sim = bass_interp.CoreSim(nc)
        for k, v in in_maps[core].items():
            sim.tensor(k)[:] = v
        sim.simulate()
        results.append({k: np.array(sim.tensor(k))
                        for k in ("aacc", "dacc", "ceacc")})
    return results


def kernel(inputs, targets):
    in_maps = _prep_in_maps(inputs, targets)
    if os.environ.get("CEDICE_SIM"):
        results = _run_sim(in_maps)
    else:
        results = _run_hw(in_maps).results
    return _combine(results, targets)
